# revision 5
# baseline (speedup 1.0000x reference)
"""Causal multi-head attention kernel for Trainium2 (Bass/Tile), 8 NeuronCores.

Problem: x[B=4,C=2048,D=1024], Q/K[dq=64,D,H=16], V[dv=64,D,H], W[D,dv,H].
Sharding: 8 shards = (batch b, half of heads). Each core computes the partial
output sum over its 8 heads for its batch; host adds the two half-head
partials per batch. No on-device collectives.

Per-core layouts (host-prepared so every DMA is contiguous):
  xT  [128, 8*C]    xT[p, j*C + c]   = x[b, c, j*128+p]
  Q2  [128, 4*1024] per head-pair pp, 8 d-chunks of [128,128] lhsT tiles,
                    cols m<64 -> head 2pp, m>=64 -> head 2pp+1 (scale folded)
  K2  same layout, unscaled
  V8  [128, 8*512]  V8[p, j*512 + (h*64+vi)] = V[vi, j*128+p, hg+h] * sv
  Wc  [128, 4*1024] Wc[p, pp*1024 + d] = W[d, p%64, hg+2*pp+p//64] * sw
  maskT [128,128]   maskT[p, s] = 1.0 if s >= p else 0.0
Output z [C, D] partial (sum over the core's 8 heads).
"""

import math
import numpy as np

# ---------------------------------------------------------------- constants
B, C, D = 4, 2048, 1024
DQ = DV = 64
H = 16
NCORES = 8
P = 128
CQ = 512                      # query block (free dim of S^T tiles)
NJ = D // P                   # 8 d-chunks
NPAIR = 4                     # head pairs per core

_nc_cache = {}


_MAXW = 1  # this walrus build rejects instructions with >1 sem wait


def _patch_tile_tail_drain(tile_mod, bass_rust, ScopedClock):
    """Work around a walrus limit on sync waits per instruction: keep at most
    _MAXW waits on any instruction; hoist the overflow onto same-engine nops
    emitted just before it (same-engine streams are sequential, so blocking at
    an earlier nop is equivalent)."""
    if getattr(tile_mod.TileContext, "_drain_patched", False):
        return

    orig_add = tile_mod.TileContext._add_instruction

    def _add_instruction(self, inst):
        si = getattr(inst, "sync_info", None)
        if si is not None and si.on_wait and len(si.on_wait) > _MAXW:
            waits = list(si.on_wait)
            si.on_wait = waits[:_MAXW]
            overflow = waits[_MAXW:]
            for i in range(0, len(overflow), _MAXW):
                nop = bass_rust.InstNoOp(
                    name=self.nc.get_next_instruction_name(), ins=[], outs=[]
                )
                nop.engine = inst.engine
                nop.sync_info = bass_rust.SyncInfo(
                    on_wait=overflow[i : i + _MAXW], on_update=[]
                )
                orig_add(self, nop)
        orig_add(self, inst)

    def _drain_and_barrier(self, tick_clock, wait_clock):
        nc = self.nc
        drain_inst = nc.sync.drain()
        wait_clock.add_sem_waits(
            drain_inst.ins, ScopedClock({None: tick_clock.global_clock})
        )
        si = drain_inst.ins.sync_info
        waits = list(si.on_wait) if si is not None and si.on_wait else []
        if len(waits) > 1:
            si.on_wait = waits[:1]
            for w in waits[1:]:
                extra = nc.sync.drain()
                esi = extra.ins.sync_info
                if esi is None:
                    extra.ins.sync_info = bass_rust.SyncInfo(
                        on_wait=[w], on_update=[]
                    )
                else:
                    esi.on_wait = list(esi.on_wait) + [w]
        nc.all_engine_barrier()
        popped = nc._tile_sem_poison_stack.pop()
        assert popped is self._sem_poison
        nc.clear_and_free_semaphores(list(self.sems.allocated().values()))
        nc.all_engine_barrier()

    tile_mod.TileContext._add_instruction = _add_instruction
    tile_mod.TileContext._drain_and_barrier = _drain_and_barrier
    tile_mod.TileContext._drain_patched = True


def build_nc(c_total=C):
    """Build the single-core Bass program (SPMD across 8 cores)."""
    import bass_rust
    import concourse.bass as bass
    import concourse.mybir as mybir
    import concourse.tile as tile
    from concourse.vector_clock import ScopedClock

    _patch_tile_tail_drain(tile, bass_rust, ScopedClock)

    f32 = mybir.dt.float32
    Alu = mybir.AluOpType
    Act = mybir.ActivationFunctionType

    NCQ = c_total // CQ           # query blocks
    NCK = c_total // P            # key chunks

    nc = bass.Bass()
    xT_d = nc.declare_dram_parameter("xT", [P, NJ * c_total], f32, isOutput=False)
    Q2_d = nc.declare_dram_parameter("Q2", [P, NPAIR * 1024], f32, isOutput=False)
    K2_d = nc.declare_dram_parameter("K2", [P, NPAIR * 1024], f32, isOutput=False)
    V8_d = nc.declare_dram_parameter("V8", [P, NJ * 512], f32, isOutput=False)
    Wc_d = nc.declare_dram_parameter("Wc", [P, NPAIR * 1024], f32, isOutput=False)
    mask_d = nc.declare_dram_parameter("maskT", [P, P], f32, isOutput=False)
    z_d = nc.declare_dram_parameter("z", [c_total, D], f32, isOutput=True)

    from contextlib import ExitStack

    with ExitStack() as stack:
        tc = stack.enter_context(tile.TileContext(nc))
        ep = stack.enter_context
        pool_xt = ep(tc.tile_pool(name="sb_xt", bufs=1))
        pool_v8 = ep(tc.tile_pool(name="sb_v8", bufs=1))
        pool_qk = ep(tc.tile_pool(name="sb_qk", bufs=2))
        pool_pt = ep(tc.tile_pool(name="sb_pt", bufs=3))
        pool_par = ep(tc.tile_pool(name="sb_par", bufs=2))
        pool_big = ep(tc.tile_pool(name="sb_big", bufs=1))
        pool_sml = ep(tc.tile_pool(name="sb_sml", bufs=1))
        pool_rcp = ep(tc.tile_pool(name="sb_rcp", bufs=2))
        pool_bc = ep(tc.tile_pool(name="sb_bc", bufs=1))
        pool_yt = ep(tc.tile_pool(name="sb_yt", bufs=1))
        pool_ytz = ep(tc.tile_pool(name="sb_ytz", bufs=4))
        pool_zo = ep(tc.tile_pool(name="sb_zo", bufs=2))
        pool_dram = ep(tc.tile_pool(name="dram", bufs=1, space="DRAM"))
        ps_proj = ep(tc.tile_pool(name="ps_proj", bufs=1, space="PSUM"))
        ps_s = ep(tc.tile_pool(name="ps_s", bufs=2, space="PSUM"))
        ps_y = ep(tc.tile_pool(name="ps_y", bufs=1, space="PSUM"))
        ps_den = ep(tc.tile_pool(name="ps_den", bufs=1, space="PSUM"))
        ps_bc = ep(tc.tile_pool(name="ps_bc", bufs=1, space="PSUM"))
        if True:
            # ---------------- phase 0: loads + constants
            xt = pool_xt.tile([P, NJ * c_total], f32, tag="xt")
            nc.sync.dma_start(out=xt[:], in_=xT_d[:])
            mask = pool_sml.tile([P, P], f32, tag="mask")
            nc.sync.dma_start(out=mask[:], in_=mask_d[:])
            v8p = pool_big.tile([P, NJ * 512], f32, tag="vw")
            nc.sync.dma_start(out=v8p[:], in_=V8_d[:])
            ones = pool_sml.tile([P, 64], f32, tag="ones")
            nc.vector.memset(ones[:], 1.0)

            # ---------------- phase 1: v projection, all 8 heads N-packed
            # v8[c-chunk cc][c_local, u=(h,vi)] with u width 512
            v8 = pool_v8.tile([P, NCK * 512], f32, tag="v8")
            for cc in range(NCK):
                vp = ps_proj.tile([P, 512], f32, tag="proj")
                for j in range(NJ):
                    nc.tensor.matmul(
                        vp[:],
                        lhsT=xt[:, j * c_total + cc * P : j * c_total + (cc + 1) * P],
                        rhs=v8p[:, j * 512 : (j + 1) * 512],
                        start=(j == 0),
                        stop=(j == NJ - 1),
                    )
                nc.vector.tensor_copy(v8[:, cc * 512 : (cc + 1) * 512], vp[:])

            # ---------------- phase 2: per head-pair: q/k proj + attention
            for pp in range(NPAIR):
                qw = pool_par.tile([P, 1024], f32, tag="qw")
                kw = pool_par.tile([P, 1024], f32, tag="kw")
                nc.sync.dma_start(out=qw[:], in_=Q2_d[:, pp * 1024 : (pp + 1) * 1024])
                nc.sync.dma_start(out=kw[:], in_=K2_d[:, pp * 1024 : (pp + 1) * 1024])

                qt = pool_qk.tile([P, c_total], f32, tag="qt")
                kt = pool_qk.tile([P, c_total], f32, tag="kt")
                for wt, dst in ((qw, qt), (kw, kt)):
                    for b in range(NCQ):
                        pr = ps_proj.tile([P, 512], f32, tag="proj")
                        for j in range(NJ):
                            nc.tensor.matmul(
                                pr[:],
                                lhsT=wt[:, j * P : (j + 1) * P],
                                rhs=xt[:, j * c_total + b * CQ : j * c_total + (b + 1) * CQ],
                                start=(j == 0),
                                stop=(j == NJ - 1),
                            )
                        nc.vector.tensor_copy(dst[:, b * CQ : (b + 1) * CQ], pr[:])

                # unnormalized attention, transposed-S flash style
                yt = pool_yt.tile([P, c_total], f32, tag="yt")
                for b in range(NCQ):
                    nck = 4 * b + 4          # causal: key chunks for this block
                    y_ps = ps_y.tile([P, CQ], f32, tag="y")
                    den = ps_den.tile([P, CQ], f32, tag="den")
                    for ck in range(nck):
                        s_ps = ps_s.tile([P, 1024], f32, tag="s")
                        # S^T = kT.T @ qT, both heads concurrently (row tiles)
                        nc.tensor.matmul(
                            s_ps[:, 0:512],
                            lhsT=kt[0:64, ck * P : (ck + 1) * P],
                            rhs=qt[0:64, b * CQ : (b + 1) * CQ],
                            start=True, stop=True,
                            tile_position=(0, 0),
                        )
                        nc.tensor.matmul(
                            s_ps[:, 512:1024],
                            lhsT=kt[64:128, ck * P : (ck + 1) * P],
                            rhs=qt[64:128, b * CQ : (b + 1) * CQ],
                            start=True, stop=True,
                            tile_position=(64, 0),
                        )
                        # exp for both heads in one ACT instruction
                        pt = pool_pt.tile([P, 1024], f32, tag="pt")
                        nc.scalar.activation(pt[:], s_ps[:], Act.Exp)
                        # causal masking on the diagonal chunk
                        diag = ck >= 4 * b
                        d0 = (ck - 4 * b) * P if diag else 0
                        if diag:
                            if d0 > 0:
                                nc.vector.memset(pt[:, 0:d0], 0.0)
                                nc.vector.memset(pt[:, 512 : 512 + d0], 0.0)
                            nc.vector.tensor_mul(
                                pt[:, d0 : d0 + P], pt[:, d0 : d0 + P], mask[:]
                            )
                            nc.vector.tensor_mul(
                                pt[:, 512 + d0 : 512 + d0 + P],
                                pt[:, 512 + d0 : 512 + d0 + P],
                                mask[:],
                            )
                        # y^T accumulation (col tiles: h0 -> rows 0-63, h1 -> 64-127)
                        u0 = pp * 128
                        nc.tensor.matmul(
                            y_ps[0:64, d0:CQ],
                            lhsT=v8[:, ck * 512 + u0 : ck * 512 + u0 + 64],
                            rhs=pt[:, d0:512],
                            start=(ck == 0), stop=(ck == nck - 1),
                            skip_group_check=True,
                            tile_position=(0, 0),
                        )
                        nc.tensor.matmul(
                            y_ps[64:128, d0:CQ],
                            lhsT=v8[:, ck * 512 + u0 + 64 : ck * 512 + u0 + 128],
                            rhs=pt[:, 512 + d0 : 1024],
                            start=(ck == 0), stop=(ck == nck - 1),
                            skip_group_check=True,
                            tile_position=(0, 64),
                        )
                        # softmax denominators (ones-column matmuls, col tiles)
                        nc.tensor.matmul(
                            den[0:1, :],
                            lhsT=ones[:, 0:1],
                            rhs=pt[:, 0:512],
                            start=(ck == 0), stop=(ck == nck - 1),
                            skip_group_check=True,
                            tile_position=(0, 0),
                        )
                        nc.tensor.matmul(
                            den[32:33, :],
                            lhsT=ones[:, 0:1],
                            rhs=pt[:, 512:1024],
                            start=(ck == 0), stop=(ck == nck - 1),
                            skip_group_check=True,
                            tile_position=(0, 32),
                        )
                    # normalize: yt[:, block] = y * (1/den) broadcast over rows
                    r0 = pool_rcp.tile([1, CQ], f32, tag="rcp")
                    r1 = pool_rcp.tile([1, CQ], f32, tag="rcp")
                    nc.vector.reciprocal(r0[:], den[0:1, :])
                    nc.vector.reciprocal(r1[:], den[32:33, :])
                    bc_ps = ps_bc.tile([P, CQ], f32, tag="bc")
                    nc.tensor.matmul(
                        bc_ps[0:64, :], lhsT=ones[0:1, 0:64], rhs=r0[:],
                        start=True, stop=True, skip_group_check=True,
                        tile_position=(0, 0),
                    )
                    nc.tensor.matmul(
                        bc_ps[64:128, :], lhsT=ones[0:1, 0:64], rhs=r1[:],
                        start=True, stop=True, skip_group_check=True,
                        tile_position=(0, 64),
                    )
                    bc = pool_bc.tile([P, CQ], f32, tag="bcs")
                    nc.vector.tensor_copy(bc[:], bc_ps[:])
                    nc.vector.scalar_tensor_tensor(
                        yt[:, b * CQ : (b + 1) * CQ],
                        in0=y_ps[:],
                        scalar=1.0,
                        in1=bc[:],
                        op0=Alu.mult,
                        op1=Alu.mult,
                    )
                ytd = pool_dram.tile([P, c_total], f32, tag=f"ytd{pp}", name=f"ytd{pp}")
                nc.sync.dma_start(out=ytd[:], in_=yt[:])
                if pp == 0:
                    ytds = []
                ytds.append(ytd)

            # ---------------- phase 3: output projection (contract all pairs)
            wc = pool_big.tile([P, NPAIR * 1024], f32, tag="vw")
            nc.sync.dma_start(out=wc[:], in_=Wc_d[:])
            for cg in range(NCQ):        # groups of 512 c-columns
                yts = []
                for pp in range(NPAIR):
                    t = pool_ytz.tile([P, 512], f32, tag="ytz", name=f"ytz{cg}_{pp}")
                    nc.sync.dma_start(
                        out=t[:], in_=ytds[pp][:, cg * 512 : (cg + 1) * 512]
                    )
                    yts.append(t)
                for ci in range(4):      # c-blocks of 128 inside the group
                    cc = cg * 4 + ci
                    for dd in range(2):  # d halves of 512
                        zp = ps_proj.tile([P, 512], f32, tag="proj")
                        for pp in range(NPAIR):
                            nc.tensor.matmul(
                                zp[:],
                                lhsT=yts[pp][:, ci * P : (ci + 1) * P],
                                rhs=wc[:, pp * 1024 + dd * 512 : pp * 1024 + (dd + 1) * 512],
                                start=(pp == 0),
                                stop=(pp == NPAIR - 1),
                            )
                        zo = pool_zo.tile([P, 512], f32, tag="zo")
                        nc.vector.tensor_copy(zo[:], zp[:])
                        nc.sync.dma_start(
                            out=z_d[cc * P : (cc + 1) * P, dd * 512 : (dd + 1) * 512],
                            in_=zo[:],
                        )
    return nc


# ---------------------------------------------------------------- host side

def shard_inputs(x, Q, K, V, W, c_total=C):
    """Build the per-core input maps (8 cores: (batch, head-half))."""
    x = np.ascontiguousarray(x, dtype=np.float32)
    Q = np.asarray(Q, dtype=np.float32)
    K = np.asarray(K, dtype=np.float32)
    V = np.asarray(V, dtype=np.float32)
    W = np.asarray(W, dtype=np.float32)

    scale_qk = (DQ / D) / DQ            # sq^2 / dq, folded into Q
    sv = math.sqrt(DV / D)
    sw = math.sqrt(D / DV) / H

    maskT = (np.arange(P)[None, :] >= np.arange(P)[:, None]).astype(np.float32)

    in_maps = []
    for core in range(NCORES):
        b = core // 2
        hg = (core % 2) * 8
        xb = x[b, :c_total]                                   # [C, D]
        xT = np.ascontiguousarray(
            xb.T.reshape(NJ, P, c_total).transpose(1, 0, 2).reshape(P, NJ * c_total)
        )
        # Q2/K2: per pair, [d, hh, m64] -> [128, pair*8 chunks of 128]
        def pack_qk(M, scale):
            out = np.empty((P, NPAIR * 1024), np.float32)
            for pp in range(NPAIR):
                g = M[:, :, hg + 2 * pp : hg + 2 * pp + 2]    # [64, D, 2]
                arr = g.transpose(1, 2, 0).reshape(NJ, P, 128)  # [d?,...]
                out[:, pp * 1024 : (pp + 1) * 1024] = (
                    arr.transpose(1, 0, 2).reshape(P, 1024) * scale
                )
            return out

        Q2 = pack_qk(Q, scale_qk)
        K2 = pack_qk(K, 1.0)
        Vg = V[:, :, hg : hg + 8]                              # [64, D, 8]
        V8 = np.ascontiguousarray(
            (Vg.transpose(1, 2, 0).reshape(NJ, P, 512) * sv)
            .transpose(1, 0, 2)
            .reshape(P, NJ * 512)
        )
        Wg = W[:, :, hg : hg + 8]                              # [D, 64, 8]
        Wc = np.empty((P, NPAIR * 1024), np.float32)
        for pp in range(NPAIR):
            wp = Wg[:, :, 2 * pp : 2 * pp + 2].transpose(2, 1, 0).reshape(P, D)
            Wc[:, pp * 1024 : (pp + 1) * 1024] = wp * sw
        in_maps.append(
            {
                "xT": xT,
                "Q2": np.ascontiguousarray(Q2),
                "K2": np.ascontiguousarray(K2),
                "V8": V8,
                "Wc": np.ascontiguousarray(Wc),
                "maskT": maskT,
            }
        )
    return in_maps


def kernel(x, Q, K, V, W):
    from concourse.bass_utils import run_bass_kernel_spmd

    if "nc" not in _nc_cache:
        _nc_cache["nc"] = build_nc(C)
    nc = _nc_cache["nc"]
    in_maps = shard_inputs(x, Q, K, V, W)
    res = run_bass_kernel_spmd(nc, in_maps, list(range(NCORES)))
    out = np.zeros((B, C, D), np.float32)
    for core in range(NCORES):
        out[core // 2] += res.results[core]["z"]
    return out
